# revision 32
# baseline (speedup 1.0000x reference)
"""Causal self-attention (B=2, T=2048, C=1024, H=16) on 8 TRN2 NeuronCores.

Sharding (Megatron-style, per the hint): each core owns one PAIR of heads
(2c, 2c+1) for BOTH batches.  Column-sharded W_qkv produces qT/kT/vT in
[feature, token] layout (the host feeds x pre-transposed so contraction is
always over partitions); v is re-laid-out to natural [token, d] via PE
transposes.  Attention computes S^T = k q^T with the two heads row-packed
in the PE array (K=64 each at partition offsets 0/64), exp on ACT with
the 1/sqrt(D) scale folded in (ACT runs exp ONLY, so its spline table is
loaded once), causal masking via host-precomputed multiplicative masks on
the diagonal tiles of each q-chunk, and A@V with a ones-column appended
to v (M=65) so the softmax denominators fall out of the same matmul.
All biases are added with K=1 rank-1 matmuls into PSUM; all PSUM->SBUF
copies run on DVE.  An 8-core AllToAll swaps head-shards for
token-shards, after which each core computes its [512, 1024] slab of the
output projection with the full (replicated) W_proj.  The host only
shards/transposes/casts inputs and concatenates the 8 output slabs.

Compute dtype bf16 (f32 accumulation in PSUM); I/O f32.
"""

import os
import sys
import types

import numpy as np

if "/opt/trn_rl_repo" not in sys.path:
    sys.path.insert(0, "/opt/trn_rl_repo")

# antenv.axon_hooks is missing on this image; shim it so trace=True can
# capture NTFF profiles (used by test harnesses; harmless otherwise).
if "antenv.axon_hooks" not in sys.modules:
    _hooks_mod = types.ModuleType("antenv.axon_hooks")
    _holder = {"hook": None}
    _hooks_mod.set_axon_ntff_profile_hook = lambda h: _holder.__setitem__("hook", h)
    _hooks_mod.get_axon_ntff_profile_hook = lambda: _holder["hook"]
    sys.modules["antenv.axon_hooks"] = _hooks_mod
    try:
        from trn_agent_boot.trn_boot import _ntff_profile_via_ctypes

        _hooks_mod.set_axon_ntff_profile_hook(
            _ntff_profile_via_ctypes("/opt/axon/libaxon_pjrt.so")
        )
    except Exception:
        pass

import ml_dtypes
from contextlib import ExitStack

import concourse.bacc as bacc
import concourse.tile as tile
from concourse import mybir
from concourse.bass_utils import run_bass_kernel_spmd

B, T, C, H = 2, 2048, 1024, 16
D = C // H          # 64
NCORES = 8
HP = 2              # heads per core
TT = B * T          # 4096 global (b, t) rows
NK = C // 128       # 8 contraction tiles over features
NW = TT // 512      # 8 token windows
NQ = T // 512       # 4 q-chunks per batch
SHARD = TT // NCORES  # 512 output rows per core

F32 = mybir.dt.float32
BF = mybir.dt.bfloat16
F8 = mybir.dt.float8e4

ActF = mybir.ActivationFunctionType

_CACHE = {}

LAST_EXEC_TIME_NS = None
LAST_RESULTS = None


def build_nc():
    nc = bacc.Bacc("TRN2", target_bir_lowering=False, debug=False,
                   num_devices=NCORES)

    xT = nc.declare_dram_parameter("xT", [C, TT], BF, isOutput=False)
    wqkv = nc.declare_dram_parameter("wqkv", [C, 3 * 128], BF, isOutput=False)
    wproj = nc.declare_dram_parameter("wproj", [C, C], BF, isOutput=False)
    masks = nc.declare_dram_parameter("masks", [128, 4 * 512], BF, isOutput=False)
    brows = nc.declare_dram_parameter("brows", [1, 3 * 128 + C], BF, isOutput=False)
    bcol = nc.declare_dram_parameter("bcol", [128, 3], F32, isOutput=False)
    ident = nc.declare_dram_parameter("ident", [128, 128], BF, isOutput=False)
    out = nc.declare_dram_parameter("out", [SHARD, C], F32, isOutput=True)

    with tile.TileContext(nc) as tc, ExitStack() as ctx:
        sb_x = ctx.enter_context(tc.tile_pool(name="sb_x", bufs=2))
        sb_w = ctx.enter_context(tc.tile_pool(name="sb_w", bufs=1))
        sb_qk = ctx.enter_context(tc.tile_pool(name="sb_qk", bufs=1))
        sb_v = ctx.enter_context(tc.tile_pool(name="sb_v", bufs=1))
        sb_att = ctx.enter_context(tc.tile_pool(name="sb_att", bufs=1))
        sb_y = ctx.enter_context(tc.tile_pool(name="sb_y", bufs=1))
        sb_tmp = ctx.enter_context(tc.tile_pool(name="sb_tmp", bufs=2))
        sb_out = ctx.enter_context(tc.tile_pool(name="sb_out", bufs=2))
        ps_mm = ctx.enter_context(tc.tile_pool(name="ps_mm", bufs=2, space="PSUM"))
        ps_s = ctx.enter_context(tc.tile_pool(name="ps_s", bufs=2, space="PSUM"))
        ps_y = ctx.enter_context(tc.tile_pool(name="ps_y", bufs=2, space="PSUM"))
        dram = ctx.enter_context(tc.tile_pool(name="dram", bufs=1, space="DRAM"))

        # ---- small loads first (weights, masks, biases) ----
        # sync queue: wqkv[kk] and window-0 x[kk] interleaved so the first
        # QKV accumulation chain starts as soon as pair 0 lands.
        ident_sb = sb_w.tile([128, 128], BF, tag="ident")
        nc.gpsimd.dma_start(ident_sb[:], ident[:])
        brows_sb = sb_w.tile([1, 3 * 128 + C], BF, tag="brows")
        nc.gpsimd.dma_start(brows_sb[:], brows[:])
        bcol_sb = sb_w.tile([128, 3], F32, tag="bcol")
        nc.gpsimd.dma_start(bcol_sb[:], bcol[:])
        wqkv_sb = []
        xw0 = []
        for kk in range(NK):
            t = sb_w.tile([128, 3 * 128], BF, tag=f"wqkv{kk}")
            nc.sync.dma_start(t[:], wqkv[128 * kk:128 * (kk + 1), :])
            wqkv_sb.append(t)
            tx = sb_x.tile([128, 512], BF, tag=f"xw{kk}", name=f"xw{kk}_0")
            nc.sync.dma_start(tx[:], xT[128 * kk:128 * (kk + 1), 0:512])
            xw0.append(tx)
        mask_sb = sb_w.tile([128, 4 * 512], BF, tag="mask")
        nc.sync.dma_start(mask_sb[:], masks[:])

        ones_sb = sb_w.tile([1, 512], BF, tag="ones")
        nc.vector.memset(ones_sb[:], 1.0)
        warm = sb_w.tile([1, 8], F32, tag="warm")
        nc.scalar.activation(warm[:], ones_sb[:, 0:8], ActF.Exp)

        # PE warm-up: ~3.5us of matmuls on the identity so the HAM clock
        # gate opens during the initial x-window DMA wait.
        warm_ps = ps_mm.tile([128, 512], F32, tag="mm", name="warmps")
        for i in range(28):
            nc.tensor.matmul(warm_ps[:, 0:128], ident_sb[:], ident_sb[:],
                             start=(i == 0), stop=(i == 27))

        # proj bias broadcast to all 128 rows, built once; fused into the
        # proj PSUM evacuation adds
        bias_bcast = sb_w.tile([128, C], BF, tag="bbc")
        for nn in range(C // 512):
            bps = ps_mm.tile([128, 512], F32, tag="mm", name=f"bbc{nn}")
            nc.tensor.matmul(bps[:], ones_sb[:, 0:128],
                             brows_sb[:, 384 + 512 * nn:384 + 512 * (nn + 1)],
                             start=True, stop=True)
            nc.vector.tensor_copy(bias_bcast[:, 512 * nn:512 * (nn + 1)], bps[:])

        # qT/kT/vT: [128 (=2 heads x 64 features), 4096 tokens]
        qT_sb = sb_qk.tile([128, TT], BF, tag="qT")
        kT_sb = sb_qk.tile([128, TT], BF, tag="kT")
        vT_sb = sb_qk.tile([128, TT], BF, tag="vT")
        dests = [qT_sb, kT_sb, vT_sb]
        # v natural: [128 tokens, 32 tiles x 130] = [vA(64) | 1 | vB(64) | 1]
        v_all = sb_v.tile([128, (TT // 128) * 130], BF, tag="v")
        # fill once: the per-slot ones columns survive the DMA-transposed
        # v writes, which only cover the 64-wide v blocks
        nc.vector.memset(v_all[:], 1.0)

        # ---- QKV projection, streamed over token windows ----
        def qkv_window(n):
            if n == 0:
                xw = xw0
            else:
                xw = []
                for kk in range(NK):
                    t = sb_x.tile([128, 512], BF, tag=f"xw{kk}",
                                  name=f"xw{kk}_{n}")
                    nc.sync.dma_start(t[:], xT[128 * kk:128 * (kk + 1),
                                               512 * n:512 * (n + 1)])
                    xw.append(t)
            for m in range(3):
                ps = ps_mm.tile([128, 512], F32, tag="mm", name=f"qkvps{n}_{m}")
                for kk in range(NK):
                    nc.tensor.matmul(
                        ps[:], wqkv_sb[kk][:, 128 * m:128 * (m + 1)], xw[kk][:],
                        start=(kk == 0), stop=(kk == NK - 1))
                # evacuate PSUM with the per-feature bias fused in
                nc.vector.tensor_scalar(
                    dests[m][:, 512 * n:512 * (n + 1)], ps[:],
                    bcol_sb[:, m:m + 1], None,
                    mybir.AluOpType.add)
            # transpose v window into natural layout
            for tt in range(4 * n, 4 * (n + 1)):
                tp = ps_mm.tile([128, 128], BF, tag="mm", name=f"vtp{tt}")
                nc.tensor.transpose(tp[:], vT_sb[:, 128 * tt:128 * (tt + 1)],
                                    ident_sb[:])
                nc.vector.tensor_copy(
                    v_all[:, 130 * tt:130 * (tt + 1)].rearrange(
                        "p (h c) -> p h c", c=65)[:, :, 0:64],
                    tp[:].rearrange("p (h c) -> p h c", c=64))

        # ---- attention ----
        yT_sb = sb_y.tile([128, TT], BF, tag="yT")
        # Chunked AllToAll: one collective per window-group so the exchange
        # overlaps attention compute.  Core c owns token slice
        # [64c, 64(c+1)) of EVERY 512-token window; cc buffers are
        # [8 dst cores x 128 feats, 64 tokens] per window.
        CC_GROUPS = [(0, 1), (2, 3), (4, 5), (6,), (7,)]
        cc_in = [dram.tile([NCORES * 128, 64], BF, tag=f"ccin{w}",
                           name=f"ccin{w}") for w in range(NW)]
        cc_out = [dram.tile([NCORES * 128, 64], BF, tag=f"ccout{w}",
                            name=f"ccout{w}") for w in range(NW)]
        # two attT tiles, alternated across chunks; memset once so that the
        # exp-skipped (causally invalid) columns of diagonal tiles hold
        # finite stale data for the mask-multiply to zero.
        attT_tiles = [
            sb_att.tile([128, 2 * 16 * 512], BF, tag=f"attT{i}", name=f"attT{i}")
            for i in range(2)]
        # zero the exp-skipped diagonal slots (cols 0:256 of k-tiles 4j+2,
        # 4j+3) once, so they hold finite data for the mask-multiply; all
        # later chunks either overwrite them (full exp) or skip them again.
        for p in range(2):
            for j in (p, p + 2):
                for i in (2, 3):
                    kt = 4 * j + i
                    for h in range(2):
                        nc.vector.memset(
                            attT_tiles[p][:, 8192 * h + 512 * kt:
                                          8192 * h + 512 * kt + 256], 0.0)

        def attention_chunk(b, j):
            tb = b * T
            kmax = 4 * (j + 1)
            qsl = slice(tb + 512 * j, tb + 512 * (j + 1))
            qsl_hi = slice(tb + 512 * j + 256, tb + 512 * (j + 1))
            attT = attT_tiles[(4 * b + j) % 2]
            # S^T per k-tile: one [128, 1024] psum tile (head A cols 0:512
            # in one bank, head B cols 512:1024 in the next; the row-tiled
            # head matmuls run concurrently into different banks).  ps_s
            # bufs=2 double-buffers the S->exp pipeline.  The two
            # upper-diagonal k-tiles only compute columns 256:512 (the rest
            # is causally dead).
            for kt in range(kmax):
                diag = kt >= kmax - 2
                sps = ps_s.tile([128, 1024], F32, tag="s",
                                name=f"sps{b}_{j}_{kt}")
                ksl = slice(tb + 128 * kt, tb + 128 * (kt + 1))
                for h in range(2):
                    hsl = slice(64 * h, 64 * (h + 1))
                    if diag:
                        nc.tensor.matmul(
                            sps[:, 512 * h + 256:512 * (h + 1)],
                            kT_sb[hsl, ksl], qT_sb[hsl, qsl_hi],
                            start=True, stop=True)
                    else:
                        nc.tensor.matmul(
                            sps[:, 512 * h:512 * (h + 1)],
                            kT_sb[hsl, ksl], qT_sb[hsl, qsl],
                            start=True, stop=True)
                dst4 = attT[:].rearrange("p (h s q) -> p h s q", h=2, q=512)
                src2 = sps[:].rearrange("p (h q) -> p h q", h=2)
                if diag:
                    nc.scalar.activation(dst4[:, :, kt, 256:512],
                                         src2[:, :, 256:512], ActF.Exp,
                                         scale=float(1.0 / np.sqrt(D)))
                else:
                    nc.scalar.activation(dst4[:, :, kt, :], src2, ActF.Exp,
                                         scale=float(1.0 / np.sqrt(D)))
            # causal masks on the 4 diagonal k-tiles (full width: columns
            # 0:256 of tiles 2,3 may hold stale finite exp values from an
            # earlier chunk on this attT buffer — the mask zeroes them)
            for i in range(4):
                kt = 4 * j + i
                for h in range(2):
                    a = attT[:, 8192 * h + 512 * kt:8192 * h + 512 * (kt + 1)]
                    nc.vector.tensor_mul(a, a, mask_sb[:, 512 * i:512 * (i + 1)])
            # A @ V (ones column gives the softmax denominator in row 64)
            for h in range(2):
                yps = ps_y.tile([65, 512], F32, tag="y", name=f"yps{b}_{j}_{h}")
                for kt in range(kmax):
                    nc.tensor.matmul(
                        yps[:], v_all[:, 130 * (b * 16 + kt) + 65 * h:
                                      130 * (b * 16 + kt) + 65 * (h + 1)],
                        attT[:, 8192 * h + 512 * kt:8192 * h + 512 * (kt + 1)],
                        start=(kt == 0), stop=(kt == kmax - 1))
                ysb = sb_tmp.tile([65, 512], F32, tag="ysb", name=f"ysb{b}{j}{h}")
                nc.vector.tensor_copy(ysb[:], yps[:])
                ltmp = sb_tmp.tile([1, 512], F32, tag="ltmp", name=f"lt{b}{j}{h}")
                nc.vector.tensor_copy(ltmp[:], ysb[64:65, :])
                recf = sb_tmp.tile([1, 512], F32, tag="recf", name=f"rf{b}{j}{h}")
                nc.vector.reciprocal_approx_fast(recf[:], ltmp[:])
                rec = sb_tmp.tile([1, 512], BF, tag="rec", name=f"rc{b}{j}{h}")
                nc.vector.tensor_copy(rec[:], recf[:])
                bc = ps_mm.tile([64, 512], F32, tag="mm", name=f"bc{b}{j}{h}")
                nc.tensor.matmul(bc[:], ones_sb[:, 0:64], rec[:],
                                 start=True, stop=True)
                nc.vector.tensor_mul(
                    yT_sb[64 * h:64 * (h + 1), qsl], ysb[0:64, :], bc[:])
            sh = 4 * b + j
            # one DMA: dst-core block d of cc_in[sh] gets token slice d
            nc.sync.dma_start(
                cc_in[sh][:].rearrange("(d r) c -> r d c", d=NCORES),
                yT_sb[:, 512 * sh:512 * (sh + 1)].rearrange(
                    "p (d c) -> p d c", d=NCORES))

        wproj_sb = []

        def issue_a2a(gi):
            ws = CC_GROUPS[gi]
            for w in ws:
                nc.gpsimd.collective_compute(
                    "AllToAll", mybir.AluOpType.bypass,
                    replica_groups=[list(range(NCORES))],
                    ins=[cc_in[w][:]], outs=[cc_out[w][:]])

        def proj_group(gi):
            """Project the received windows of group gi.

            Output rows (local shard order): window w contributes rows
            [64w, 64(w+1)); groups cover contiguous row blocks."""
            ws = CC_GROUPS[gi]
            m = 64 * len(ws)
            r0 = 64 * ws[0]
            y_lhs = []
            for kk in range(NK):
                t = sb_tmp.tile([128, 128], BF, tag=f"ylhs{kk}",
                                name=f"ylhs{kk}_{gi}")
                for wi, w in enumerate(ws):
                    eng = nc.sync if kk % 2 == 0 else nc.scalar
                    eng.dma_start(
                        t[:, 64 * wi:64 * (wi + 1)],
                        cc_out[w][128 * kk:128 * (kk + 1), :])
                y_lhs.append(t)
            pss = []
            for nn in range(C // 512):
                ps = ps_mm.tile([128, 512], F32, tag="mm", name=f"prj{gi}_{nn}")
                pss.append(ps)
            for kk in range(NK):
                for nn in range(C // 512):
                    nc.tensor.matmul(
                        pss[nn][0:m, :], y_lhs[kk][:, 0:m],
                        wproj_sb[kk][:, 512 * nn:512 * (nn + 1)],
                        start=(kk == 0), stop=(kk == NK - 1))
            for nn in range(C // 512):
                o = sb_out.tile([128, 512], F32, tag="o", name=f"o{gi}_{nn}")
                nc.vector.tensor_add(o[0:m, :], pss[nn][0:m, :],
                                     bias_bcast[0:m, 512 * nn:512 * (nn + 1)])
                nc.sync.dma_start(
                    out[r0:r0 + m, 512 * nn:512 * (nn + 1)], o[0:m, :])

        # batch-0 windows, then batch-0 attention interleaved (in priority)
        # with batch-1 windows, then batch-1 attention.  A2A chunks fire as
        # their windows complete (the rendezvous with late-launched cores
        # happens on the CC stream, overlapping compute); ALL proj groups
        # are emitted last so no compute engine ever blocks mid-stream on
        # collective output.
        for n in range(4):
            qkv_window(n)
            if n == 1:
                # wproj needed once proj groups start; emitted here so its
                # DMA fills bandwidth after the first x windows.
                for kk in range(NK):
                    t = sb_w.tile([128, C], BF, tag=f"wproj{kk}")
                    nc.sync.dma_start(t[:], wproj[128 * kk:128 * (kk + 1), :])
                    wproj_sb.append(t)
        for j in range(NQ):
            attention_chunk(0, j)
            # batch-1 window emitted here so its matmuls fill the PE gaps
            # left by the S->exp PSUM ping-pong
            qkv_window(4 + j)
            if j == 1:
                issue_a2a(0)
            if j == 3:
                issue_a2a(1)
        for j in range(NQ):
            attention_chunk(1, j)
            if j == 1:
                issue_a2a(2)
            if j == 2:
                issue_a2a(3)
            if j == 3:
                issue_a2a(4)
        for gi in range(len(CC_GROUPS)):
            proj_group(gi)

    nc.compile()
    return nc


def _host_inputs(x, W_qkv, b_qkv, W_proj, b_proj):
    """Shard/layout/cast inputs for each core."""
    bf = ml_dtypes.bfloat16
    xT = np.ascontiguousarray(
        x.reshape(TT, C).T).astype(bf)                    # [C, TT]
    wproj = W_proj.astype(bf)                             # [C, C]
    kk_idx = np.arange(128)[:, None]
    qq_idx = np.arange(512)[None, :]
    masks = np.concatenate(
        [(128 * i + kk_idx <= qq_idx) for i in range(4)],
        axis=1).astype(bf)                                # [128, 2048]
    ident = np.eye(128).astype(bf)

    in_maps = []
    for c in range(NCORES):
        h0 = HP * c * D
        cols = slice(h0, h0 + HP * D)                     # 128 cols
        wq = W_qkv[:, cols]
        wk = W_qkv[:, C:][:, cols]
        wv = W_qkv[:, 2 * C:][:, cols]
        wqkv = np.concatenate([wq, wk, wv], axis=1).astype(bf)   # [C, 384]
        bq3 = np.concatenate([b_qkv[cols], b_qkv[C:][cols], b_qkv[2 * C:][cols]])
        brows = np.concatenate([bq3, b_proj])[None, :].astype(bf)  # [1, 1408]
        in_maps.append({
            "xT": xT, "wqkv": wqkv, "wproj": wproj,
            "masks": masks, "brows": brows, "bcol": np.ascontiguousarray(bq3.reshape(3, 128).T).astype(np.float32),
            "ident": ident,
        })
    return in_maps


def kernel(x, W_qkv, b_qkv, W_proj, b_proj):
    global LAST_EXEC_TIME_NS, LAST_RESULTS
    x = np.asarray(x, dtype=np.float32)
    W_qkv = np.asarray(W_qkv, dtype=np.float32)
    b_qkv = np.asarray(b_qkv, dtype=np.float32)
    W_proj = np.asarray(W_proj, dtype=np.float32)
    b_proj = np.asarray(b_proj, dtype=np.float32)

    if "nc" not in _CACHE:
        _CACHE["nc"] = build_nc()
    nc = _CACHE["nc"]

    in_maps = _host_inputs(x, W_qkv, b_qkv, W_proj, b_proj)
    trace = os.environ.get("TRN_KERNEL_TRACE", "0") == "1"
    kw = {}
    if os.environ.get("TRN_KERNEL_TRACE_ALL", "0") == "1":
        kw["trace_cores"] = list(range(NCORES))
    res = run_bass_kernel_spmd(nc, in_maps, core_ids=list(range(NCORES)),
                               trace=trace, **kw)
    LAST_EXEC_TIME_NS = res.exec_time_ns
    LAST_RESULTS = res
    # core c's out rows [64w, 64(w+1)) = global rows [512w+64c, 512w+64(c+1))
    full = np.empty((TT, C), dtype=np.float32)
    for c in range(NCORES):
        oc = res.results[c]["out"]
        for w in range(NW):
            full[512 * w + 64 * c:512 * w + 64 * (c + 1)] = \
                oc[64 * w:64 * (w + 1)]
    return full.reshape(B, T, C).astype(np.float32)



# revision 34
# speedup vs baseline: 1.0126x; 1.0126x over previous
"""Causal self-attention (B=2, T=2048, C=1024, H=16) on 8 TRN2 NeuronCores.

Sharding (Megatron-style, per the hint): each core owns one PAIR of heads
(2c, 2c+1) for BOTH batches.  Column-sharded W_qkv produces qT/kT/vT in
[feature, token] layout (the host feeds x pre-transposed so contraction is
always over partitions); v is re-laid-out to natural [token, d] via PE
transposes.  Attention computes S^T = k q^T with the two heads row-packed
in the PE array (K=64 each at partition offsets 0/64), exp on ACT with
the 1/sqrt(D) scale folded in (ACT runs exp ONLY, so its spline table is
loaded once), causal masking via host-precomputed multiplicative masks on
the diagonal tiles of each q-chunk, and A@V with a ones-column appended
to v (M=65) so the softmax denominators fall out of the same matmul.
All biases are added with K=1 rank-1 matmuls into PSUM; all PSUM->SBUF
copies run on DVE.  An 8-core AllToAll swaps head-shards for
token-shards, after which each core computes its [512, 1024] slab of the
output projection with the full (replicated) W_proj.  The host only
shards/transposes/casts inputs and concatenates the 8 output slabs.

Compute dtype bf16 (f32 accumulation in PSUM); I/O f32.
"""

import os
import sys
import types

import numpy as np

if "/opt/trn_rl_repo" not in sys.path:
    sys.path.insert(0, "/opt/trn_rl_repo")

# antenv.axon_hooks is missing on this image; shim it so trace=True can
# capture NTFF profiles (used by test harnesses; harmless otherwise).
if "antenv.axon_hooks" not in sys.modules:
    _hooks_mod = types.ModuleType("antenv.axon_hooks")
    _holder = {"hook": None}
    _hooks_mod.set_axon_ntff_profile_hook = lambda h: _holder.__setitem__("hook", h)
    _hooks_mod.get_axon_ntff_profile_hook = lambda: _holder["hook"]
    sys.modules["antenv.axon_hooks"] = _hooks_mod
    try:
        from trn_agent_boot.trn_boot import _ntff_profile_via_ctypes

        _hooks_mod.set_axon_ntff_profile_hook(
            _ntff_profile_via_ctypes("/opt/axon/libaxon_pjrt.so")
        )
    except Exception:
        pass

import ml_dtypes
from contextlib import ExitStack

import concourse.bacc as bacc
import concourse.tile as tile
from concourse import mybir
from concourse.bass_utils import run_bass_kernel_spmd

B, T, C, H = 2, 2048, 1024, 16
D = C // H          # 64
NCORES = 8
HP = 2              # heads per core
TT = B * T          # 4096 global (b, t) rows
NK = C // 128       # 8 contraction tiles over features
NW = TT // 512      # 8 token windows
NQ = T // 512       # 4 q-chunks per batch
SHARD = TT // NCORES  # 512 output rows per core

F32 = mybir.dt.float32
BF = mybir.dt.bfloat16
F8 = mybir.dt.float8e4

ActF = mybir.ActivationFunctionType

_CACHE = {}

LAST_EXEC_TIME_NS = None
LAST_RESULTS = None


def build_nc():
    nc = bacc.Bacc("TRN2", target_bir_lowering=False, debug=False,
                   num_devices=NCORES)

    xT = nc.declare_dram_parameter("xT", [C, TT], BF, isOutput=False)
    wqkv = nc.declare_dram_parameter("wqkv", [C, 3 * 128], BF, isOutput=False)
    wproj = nc.declare_dram_parameter("wproj", [C, C], BF, isOutput=False)
    masks = nc.declare_dram_parameter("masks", [128, 4 * 512], BF, isOutput=False)
    brows = nc.declare_dram_parameter("brows", [1, 3 * 128 + C], BF, isOutput=False)
    bcol = nc.declare_dram_parameter("bcol", [128, 3], F32, isOutput=False)
    ident = nc.declare_dram_parameter("ident", [128, 128], BF, isOutput=False)
    out = nc.declare_dram_parameter("out", [SHARD, C], F32, isOutput=True)

    with tile.TileContext(nc) as tc, ExitStack() as ctx:
        sb_x = ctx.enter_context(tc.tile_pool(name="sb_x", bufs=2))
        sb_w = ctx.enter_context(tc.tile_pool(name="sb_w", bufs=1))
        sb_qk = ctx.enter_context(tc.tile_pool(name="sb_qk", bufs=1))
        sb_v = ctx.enter_context(tc.tile_pool(name="sb_v", bufs=1))
        sb_att = ctx.enter_context(tc.tile_pool(name="sb_att", bufs=1))
        sb_y = ctx.enter_context(tc.tile_pool(name="sb_y", bufs=1))
        sb_tmp = ctx.enter_context(tc.tile_pool(name="sb_tmp", bufs=2))
        sb_out = ctx.enter_context(tc.tile_pool(name="sb_out", bufs=2))
        ps_mm = ctx.enter_context(tc.tile_pool(name="ps_mm", bufs=2, space="PSUM"))
        ps_s = ctx.enter_context(tc.tile_pool(name="ps_s", bufs=1, space="PSUM"))
        ps_y = ctx.enter_context(tc.tile_pool(name="ps_y", bufs=2, space="PSUM"))
        dram = ctx.enter_context(tc.tile_pool(name="dram", bufs=1, space="DRAM"))

        # ---- small loads first (weights, masks, biases) ----
        # sync queue: wqkv[kk] and window-0 x[kk] interleaved so the first
        # QKV accumulation chain starts as soon as pair 0 lands.
        ident_sb = sb_w.tile([128, 128], BF, tag="ident")
        nc.gpsimd.dma_start(ident_sb[:], ident[:])
        brows_sb = sb_w.tile([1, 3 * 128 + C], BF, tag="brows")
        nc.gpsimd.dma_start(brows_sb[:], brows[:])
        bcol_sb = sb_w.tile([128, 3], F32, tag="bcol")
        nc.gpsimd.dma_start(bcol_sb[:], bcol[:])
        wqkv_sb = []
        xw0 = []
        for kk in range(NK):
            t = sb_w.tile([128, 3 * 128], BF, tag=f"wqkv{kk}")
            nc.sync.dma_start(t[:], wqkv[128 * kk:128 * (kk + 1), :])
            wqkv_sb.append(t)
            tx = sb_x.tile([128, 512], BF, tag=f"xw{kk}", name=f"xw{kk}_0")
            nc.sync.dma_start(tx[:], xT[128 * kk:128 * (kk + 1), 0:512])
            xw0.append(tx)
        mask_sb = sb_w.tile([128, 4 * 512], BF, tag="mask")
        nc.sync.dma_start(mask_sb[:], masks[:])

        ones_sb = sb_w.tile([1, 512], BF, tag="ones")
        nc.vector.memset(ones_sb[:], 1.0)
        warm = sb_w.tile([1, 8], F32, tag="warm")
        nc.scalar.activation(warm[:], ones_sb[:, 0:8], ActF.Exp)

        # PE warm-up: ~3.5us of matmuls on the identity so the HAM clock
        # gate opens during the initial x-window DMA wait.
        warm_ps = ps_mm.tile([128, 512], F32, tag="mm", name="warmps")
        for i in range(28):
            nc.tensor.matmul(warm_ps[:, 0:128], ident_sb[:], ident_sb[:],
                             start=(i == 0), stop=(i == 27))

        # proj bias broadcast to all 128 rows, built once; fused into the
        # proj PSUM evacuation adds
        bias_bcast = sb_w.tile([128, C], BF, tag="bbc")
        for nn in range(C // 512):
            bps = ps_mm.tile([128, 512], F32, tag="mm", name=f"bbc{nn}")
            nc.tensor.matmul(bps[:], ones_sb[:, 0:128],
                             brows_sb[:, 384 + 512 * nn:384 + 512 * (nn + 1)],
                             start=True, stop=True)
            nc.vector.tensor_copy(bias_bcast[:, 512 * nn:512 * (nn + 1)], bps[:])

        # qT/kT/vT: [128 (=2 heads x 64 features), 4096 tokens]
        qT_sb = sb_qk.tile([128, TT], BF, tag="qT")
        kT_sb = sb_qk.tile([128, TT], BF, tag="kT")
        vT_sb = sb_qk.tile([128, TT], BF, tag="vT")
        dests = [qT_sb, kT_sb, vT_sb]
        # v natural: [128 tokens, 32 tiles x 130] = [vA(64) | 1 | vB(64) | 1]
        v_all = sb_v.tile([128, (TT // 128) * 130], BF, tag="v")
        # fill once: the per-slot ones columns survive the DMA-transposed
        # v writes, which only cover the 64-wide v blocks
        nc.vector.memset(v_all[:], 1.0)

        # ---- QKV projection, streamed over token windows ----
        def qkv_window(n):
            if n == 0:
                xw = xw0
            else:
                xw = []
                for kk in range(NK):
                    t = sb_x.tile([128, 512], BF, tag=f"xw{kk}",
                                  name=f"xw{kk}_{n}")
                    nc.sync.dma_start(t[:], xT[128 * kk:128 * (kk + 1),
                                               512 * n:512 * (n + 1)])
                    xw.append(t)
            for m in range(3):
                ps = ps_mm.tile([128, 512], F32, tag="mm", name=f"qkvps{n}_{m}")
                for kk in range(NK):
                    nc.tensor.matmul(
                        ps[:], wqkv_sb[kk][:, 128 * m:128 * (m + 1)], xw[kk][:],
                        start=(kk == 0), stop=(kk == NK - 1))
                # evacuate PSUM with the per-feature bias fused in
                nc.vector.tensor_scalar(
                    dests[m][:, 512 * n:512 * (n + 1)], ps[:],
                    bcol_sb[:, m:m + 1], None,
                    mybir.AluOpType.add)
            # transpose v window into natural layout
            for tt in range(4 * n, 4 * (n + 1)):
                tp = ps_mm.tile([128, 128], BF, tag="mm", name=f"vtp{tt}")
                nc.tensor.transpose(tp[:], vT_sb[:, 128 * tt:128 * (tt + 1)],
                                    ident_sb[:])
                nc.vector.tensor_copy(
                    v_all[:, 130 * tt:130 * (tt + 1)].rearrange(
                        "p (h c) -> p h c", c=65)[:, :, 0:64],
                    tp[:].rearrange("p (h c) -> p h c", c=64))

        # ---- attention ----
        yT_sb = sb_y.tile([128, TT], BF, tag="yT")
        # Chunked AllToAll: one collective per window-group so the exchange
        # overlaps attention compute.  Core c owns token slice
        # [64c, 64(c+1)) of EVERY 512-token window; cc buffers are
        # [8 dst cores x 128 feats, 64 tokens] per window.
        CC_GROUPS = [(0, 1), (2, 3), (4, 5), (6,), (7,)]
        cc_in = [dram.tile([NCORES * 128, 64], BF, tag=f"ccin{w}",
                           name=f"ccin{w}") for w in range(NW)]
        cc_out = [dram.tile([NCORES * 128, 64], BF, tag=f"ccout{w}",
                            name=f"ccout{w}") for w in range(NW)]
        # two attT tiles, alternated across chunks; memset once so that the
        # exp-skipped (causally invalid) columns of diagonal tiles hold
        # finite stale data for the mask-multiply to zero.
        attT_tiles = [
            sb_att.tile([128, 2 * 16 * 512], BF, tag=f"attT{i}", name=f"attT{i}")
            for i in range(2)]
        # zero the exp-skipped diagonal slots (cols 0:256 of k-tiles 4j+2,
        # 4j+3) once, so they hold finite data for the mask-multiply; all
        # later chunks either overwrite them (full exp) or skip them again.
        for p in range(2):
            for j in (p, p + 2):
                for i in (2, 3):
                    kt = 4 * j + i
                    for h in range(2):
                        nc.vector.memset(
                            attT_tiles[p][:, 8192 * h + 512 * kt:
                                          8192 * h + 512 * kt + 256], 0.0)

        def attention_chunk(b, j):
            tb = b * T
            kmax = 4 * (j + 1)
            qsl = slice(tb + 512 * j, tb + 512 * (j + 1))
            qsl_hi = slice(tb + 512 * j + 256, tb + 512 * (j + 1))
            attT = attT_tiles[(4 * b + j) % 2]
            # S^T in groups of 2 k-tiles x 2 heads -> one [128, 2048] psum
            # tile; exp per group.  The last group's two upper-diagonal
            # k-tiles only compute columns 256:512 (the rest is dead).
            # A@V matmuls for group g-1 are emitted right after group g's
            # S matmuls so the PE has ready work while exp(g) drains the
            # psum tile (ps_s is single-buffered).  Masks for the diagonal
            # tiles run right after their group's exp.
            yps = [ps_y.tile([65, 512], F32, tag="y", name=f"yps{b}_{j}_{h}")
                   for h in range(2)]

            def s_group(g):
                diag = g == kmax // 2 - 1
                sps = ps_s.tile([128, 2048], F32, tag="s", name=f"sps{b}_{j}_{g}")
                for i in range(2):
                    kt = 2 * g + i
                    ksl = slice(tb + 128 * kt, tb + 128 * (kt + 1))
                    for h in range(2):
                        hsl = slice(64 * h, 64 * (h + 1))
                        if diag:
                            nc.tensor.matmul(
                                sps[:, 1024 * h + 512 * i + 256:
                                    1024 * h + 512 * (i + 1)],
                                kT_sb[hsl, ksl], qT_sb[hsl, qsl_hi],
                                start=True, stop=True)
                        else:
                            nc.tensor.matmul(
                                sps[:, 1024 * h + 512 * i:
                                    1024 * h + 512 * (i + 1)],
                                kT_sb[hsl, ksl], qT_sb[hsl, qsl],
                                start=True, stop=True)
                if diag:
                    dst = attT[:].rearrange("p (h s q) -> p h s q",
                                            h=2, q=512)[
                        :, :, 2 * g:2 * (g + 1), 256:512]
                    srcv = sps[:].rearrange("p (h s q) -> p h s q",
                                            h=2, q=512)[:, :, :, 256:512]
                    nc.scalar.activation(dst, srcv, ActF.Exp,
                                         scale=float(1.0 / np.sqrt(D)))
                else:
                    dst = attT[:].rearrange("p (h s) -> p h s", h=2)[
                        :, :, 512 * 2 * g:512 * 2 * (g + 1)]
                    nc.scalar.activation(dst, sps[:].rearrange(
                        "p (h s) -> p h s", h=2), ActF.Exp,
                        scale=float(1.0 / np.sqrt(D)))
                # masks for this group's tiles if they are diagonal-block
                for i in range(2):
                    kt = 2 * g + i
                    di = kt - 4 * j
                    if di >= 0:
                        for h in range(2):
                            a = attT[:, 8192 * h + 512 * kt:
                                     8192 * h + 512 * (kt + 1)]
                            nc.vector.tensor_mul(
                                a, a, mask_sb[:, 512 * di:512 * (di + 1)])

            def av_group(g):
                for i in range(2):
                    kt = 2 * g + i
                    for h in range(2):
                        nc.tensor.matmul(
                            yps[h][:], v_all[:, 130 * (b * 16 + kt) + 65 * h:
                                             130 * (b * 16 + kt) + 65 * (h + 1)],
                            attT[:, 8192 * h + 512 * kt:
                                 8192 * h + 512 * (kt + 1)],
                            start=(kt == 0), stop=(kt == kmax - 1))

            s_group(0)
            for g in range(1, kmax // 2):
                s_group(g)
                av_group(g - 1)
            av_group(kmax // 2 - 1)

            for h in range(2):
                ysb = sb_tmp.tile([65, 512], F32, tag="ysb", name=f"ysb{b}{j}{h}")
                nc.vector.tensor_copy(ysb[:], yps[h][:])
                ltmp = sb_tmp.tile([1, 512], F32, tag="ltmp", name=f"lt{b}{j}{h}")
                nc.vector.tensor_copy(ltmp[:], ysb[64:65, :])
                recf = sb_tmp.tile([1, 512], F32, tag="recf", name=f"rf{b}{j}{h}")
                nc.vector.reciprocal_approx_fast(recf[:], ltmp[:])
                rec = sb_tmp.tile([1, 512], BF, tag="rec", name=f"rc{b}{j}{h}")
                nc.vector.tensor_copy(rec[:], recf[:])
                bc = ps_mm.tile([64, 512], F32, tag="mm", name=f"bc{b}{j}{h}")
                nc.tensor.matmul(bc[:], ones_sb[:, 0:64], rec[:],
                                 start=True, stop=True)
                nc.vector.tensor_mul(
                    yT_sb[64 * h:64 * (h + 1), qsl], ysb[0:64, :], bc[:])
            sh = 4 * b + j
            # one DMA: dst-core block d of cc_in[sh] gets token slice d
            nc.sync.dma_start(
                cc_in[sh][:].rearrange("(d r) c -> r d c", d=NCORES),
                yT_sb[:, 512 * sh:512 * (sh + 1)].rearrange(
                    "p (d c) -> p d c", d=NCORES))

        wproj_sb = []

        def issue_a2a(gi):
            ws = CC_GROUPS[gi]
            for w in ws:
                nc.gpsimd.collective_compute(
                    "AllToAll", mybir.AluOpType.bypass,
                    replica_groups=[list(range(NCORES))],
                    ins=[cc_in[w][:]], outs=[cc_out[w][:]])

        def proj_group(gi):
            """Project the received windows of group gi.

            Output rows (local shard order): window w contributes rows
            [64w, 64(w+1)); groups cover contiguous row blocks."""
            ws = CC_GROUPS[gi]
            m = 64 * len(ws)
            r0 = 64 * ws[0]
            y_lhs = []
            for kk in range(NK):
                t = sb_tmp.tile([128, 128], BF, tag=f"ylhs{kk}",
                                name=f"ylhs{kk}_{gi}")
                for wi, w in enumerate(ws):
                    eng = nc.sync if kk % 2 == 0 else nc.scalar
                    eng.dma_start(
                        t[:, 64 * wi:64 * (wi + 1)],
                        cc_out[w][128 * kk:128 * (kk + 1), :])
                y_lhs.append(t)
            pss = []
            for nn in range(C // 512):
                ps = ps_mm.tile([128, 512], F32, tag="mm", name=f"prj{gi}_{nn}")
                pss.append(ps)
            for kk in range(NK):
                for nn in range(C // 512):
                    nc.tensor.matmul(
                        pss[nn][0:m, :], y_lhs[kk][:, 0:m],
                        wproj_sb[kk][:, 512 * nn:512 * (nn + 1)],
                        start=(kk == 0), stop=(kk == NK - 1))
            for nn in range(C // 512):
                o = sb_out.tile([128, 512], F32, tag="o", name=f"o{gi}_{nn}")
                nc.vector.tensor_add(o[0:m, :], pss[nn][0:m, :],
                                     bias_bcast[0:m, 512 * nn:512 * (nn + 1)])
                nc.sync.dma_start(
                    out[r0:r0 + m, 512 * nn:512 * (nn + 1)], o[0:m, :])

        # batch-0 windows, then batch-0 attention interleaved (in priority)
        # with batch-1 windows, then batch-1 attention.  A2A chunks fire as
        # their windows complete (the rendezvous with late-launched cores
        # happens on the CC stream, overlapping compute); ALL proj groups
        # are emitted last so no compute engine ever blocks mid-stream on
        # collective output.
        for n in range(4):
            qkv_window(n)
            if n == 1:
                # wproj needed once proj groups start; emitted here so its
                # DMA fills bandwidth after the first x windows.
                for kk in range(NK):
                    t = sb_w.tile([128, C], BF, tag=f"wproj{kk}")
                    nc.sync.dma_start(t[:], wproj[128 * kk:128 * (kk + 1), :])
                    wproj_sb.append(t)
        for j in range(NQ):
            attention_chunk(0, j)
            # batch-1 window emitted here so its matmuls fill the PE gaps
            # left by the S->exp PSUM ping-pong
            qkv_window(4 + j)
            if j == 1:
                issue_a2a(0)
            if j == 3:
                issue_a2a(1)
        for j in range(NQ):
            attention_chunk(1, j)
            if j == 1:
                issue_a2a(2)
            if j == 2:
                issue_a2a(3)
            if j == 3:
                issue_a2a(4)
        for gi in range(len(CC_GROUPS)):
            proj_group(gi)

    nc.compile()
    return nc


def _host_inputs(x, W_qkv, b_qkv, W_proj, b_proj):
    """Shard/layout/cast inputs for each core."""
    bf = ml_dtypes.bfloat16
    xT = np.ascontiguousarray(
        x.reshape(TT, C).T).astype(bf)                    # [C, TT]
    wproj = W_proj.astype(bf)                             # [C, C]
    kk_idx = np.arange(128)[:, None]
    qq_idx = np.arange(512)[None, :]
    masks = np.concatenate(
        [(128 * i + kk_idx <= qq_idx) for i in range(4)],
        axis=1).astype(bf)                                # [128, 2048]
    ident = np.eye(128).astype(bf)

    in_maps = []
    for c in range(NCORES):
        h0 = HP * c * D
        cols = slice(h0, h0 + HP * D)                     # 128 cols
        wq = W_qkv[:, cols]
        wk = W_qkv[:, C:][:, cols]
        wv = W_qkv[:, 2 * C:][:, cols]
        wqkv = np.concatenate([wq, wk, wv], axis=1).astype(bf)   # [C, 384]
        bq3 = np.concatenate([b_qkv[cols], b_qkv[C:][cols], b_qkv[2 * C:][cols]])
        brows = np.concatenate([bq3, b_proj])[None, :].astype(bf)  # [1, 1408]
        in_maps.append({
            "xT": xT, "wqkv": wqkv, "wproj": wproj,
            "masks": masks, "brows": brows, "bcol": np.ascontiguousarray(bq3.reshape(3, 128).T).astype(np.float32),
            "ident": ident,
        })
    return in_maps


def kernel(x, W_qkv, b_qkv, W_proj, b_proj):
    global LAST_EXEC_TIME_NS, LAST_RESULTS
    x = np.asarray(x, dtype=np.float32)
    W_qkv = np.asarray(W_qkv, dtype=np.float32)
    b_qkv = np.asarray(b_qkv, dtype=np.float32)
    W_proj = np.asarray(W_proj, dtype=np.float32)
    b_proj = np.asarray(b_proj, dtype=np.float32)

    if "nc" not in _CACHE:
        _CACHE["nc"] = build_nc()
    nc = _CACHE["nc"]

    in_maps = _host_inputs(x, W_qkv, b_qkv, W_proj, b_proj)
    trace = os.environ.get("TRN_KERNEL_TRACE", "0") == "1"
    kw = {}
    if os.environ.get("TRN_KERNEL_TRACE_ALL", "0") == "1":
        kw["trace_cores"] = list(range(NCORES))
    res = run_bass_kernel_spmd(nc, in_maps, core_ids=list(range(NCORES)),
                               trace=trace, **kw)
    LAST_EXEC_TIME_NS = res.exec_time_ns
    LAST_RESULTS = res
    # core c's out rows [64w, 64(w+1)) = global rows [512w+64c, 512w+64(c+1))
    full = np.empty((TT, C), dtype=np.float32)
    for c in range(NCORES):
        oc = res.results[c]["out"]
        for w in range(NW):
            full[512 * w + 64 * c:512 * w + 64 * (c + 1)] = \
                oc[64 * w:64 * (w + 1)]
    return full.reshape(B, T, C).astype(np.float32)



# revision 36
# speedup vs baseline: 1.0593x; 1.0462x over previous
"""Causal self-attention (B=2, T=2048, C=1024, H=16) on 8 TRN2 NeuronCores.

Sharding (Megatron-style, per the hint): each core owns one PAIR of heads
(2c, 2c+1) for BOTH batches.  Column-sharded W_qkv produces qT/kT/vT in
[feature, token] layout (the host feeds x pre-transposed so contraction is
always over partitions); v is re-laid-out to natural [token, d] via PE
transposes.  Attention computes S^T = k q^T with the two heads row-packed
in the PE array (K=64 each at partition offsets 0/64), exp on ACT with
the 1/sqrt(D) scale folded in (ACT runs exp ONLY, so its spline table is
loaded once), causal masking via host-precomputed multiplicative masks on
the diagonal tiles of each q-chunk, and A@V with a ones-column appended
to v (M=65) so the softmax denominators fall out of the same matmul.
All biases are added with K=1 rank-1 matmuls into PSUM; all PSUM->SBUF
copies run on DVE.  An 8-core AllToAll swaps head-shards for
token-shards, after which each core computes its [512, 1024] slab of the
output projection with the full (replicated) W_proj.  The host only
shards/transposes/casts inputs and concatenates the 8 output slabs.

Compute dtype bf16 (f32 accumulation in PSUM); I/O f32.
"""

import os
import sys
import types

import numpy as np

if "/opt/trn_rl_repo" not in sys.path:
    sys.path.insert(0, "/opt/trn_rl_repo")

# antenv.axon_hooks is missing on this image; shim it so trace=True can
# capture NTFF profiles (used by test harnesses; harmless otherwise).
if "antenv.axon_hooks" not in sys.modules:
    _hooks_mod = types.ModuleType("antenv.axon_hooks")
    _holder = {"hook": None}
    _hooks_mod.set_axon_ntff_profile_hook = lambda h: _holder.__setitem__("hook", h)
    _hooks_mod.get_axon_ntff_profile_hook = lambda: _holder["hook"]
    sys.modules["antenv.axon_hooks"] = _hooks_mod
    try:
        from trn_agent_boot.trn_boot import _ntff_profile_via_ctypes

        _hooks_mod.set_axon_ntff_profile_hook(
            _ntff_profile_via_ctypes("/opt/axon/libaxon_pjrt.so")
        )
    except Exception:
        pass

import ml_dtypes
from contextlib import ExitStack

import concourse.bacc as bacc
import concourse.tile as tile
from concourse import mybir
from concourse.bass_utils import run_bass_kernel_spmd

B, T, C, H = 2, 2048, 1024, 16
D = C // H          # 64
NCORES = 8
HP = 2              # heads per core
TT = B * T          # 4096 global (b, t) rows
NK = C // 128       # 8 contraction tiles over features
NW = TT // 512      # 8 token windows
NQ = T // 512       # 4 q-chunks per batch
SHARD = TT // NCORES  # 512 output rows per core

F32 = mybir.dt.float32
BF = mybir.dt.bfloat16
F8 = mybir.dt.float8e4

ActF = mybir.ActivationFunctionType

_CACHE = {}

LAST_EXEC_TIME_NS = None
LAST_RESULTS = None


def build_nc():
    nc = bacc.Bacc("TRN2", target_bir_lowering=False, debug=False,
                   num_devices=NCORES)

    xT = nc.declare_dram_parameter("xT", [C, TT], BF, isOutput=False)
    wqkv = nc.declare_dram_parameter("wqkv", [C, 3 * 128], BF, isOutput=False)
    wproj = nc.declare_dram_parameter("wproj", [C, C], BF, isOutput=False)
    masks = nc.declare_dram_parameter("masks", [128, 4 * 512], BF, isOutput=False)
    brows = nc.declare_dram_parameter("brows", [1, 3 * 128 + C], BF, isOutput=False)
    bcol = nc.declare_dram_parameter("bcol", [128, 3], F32, isOutput=False)
    ident = nc.declare_dram_parameter("ident", [128, 128], BF, isOutput=False)
    out = nc.declare_dram_parameter("out", [SHARD, C], F32, isOutput=True)

    with tile.TileContext(nc) as tc, ExitStack() as ctx:
        sb_x = ctx.enter_context(tc.tile_pool(name="sb_x", bufs=2))
        sb_w = ctx.enter_context(tc.tile_pool(name="sb_w", bufs=1))
        sb_qk = ctx.enter_context(tc.tile_pool(name="sb_qk", bufs=1))
        sb_v = ctx.enter_context(tc.tile_pool(name="sb_v", bufs=1))
        sb_att = ctx.enter_context(tc.tile_pool(name="sb_att", bufs=1))
        sb_y = ctx.enter_context(tc.tile_pool(name="sb_y", bufs=1))
        sb_tmp = ctx.enter_context(tc.tile_pool(name="sb_tmp", bufs=2))
        sb_out = ctx.enter_context(tc.tile_pool(name="sb_out", bufs=2))
        ps_mm = ctx.enter_context(tc.tile_pool(name="ps_mm", bufs=2, space="PSUM"))
        ps_s = ctx.enter_context(tc.tile_pool(name="ps_s", bufs=1, space="PSUM"))
        ps_y = ctx.enter_context(tc.tile_pool(name="ps_y", bufs=2, space="PSUM"))
        dram = ctx.enter_context(tc.tile_pool(name="dram", bufs=1, space="DRAM"))

        # ---- small loads first (weights, masks, biases) ----
        # sync queue: wqkv[kk] and window-0 x[kk] interleaved so the first
        # QKV accumulation chain starts as soon as pair 0 lands.
        ident_sb = sb_w.tile([128, 128], BF, tag="ident")
        nc.gpsimd.dma_start(ident_sb[:], ident[:])
        brows_sb = sb_w.tile([1, 3 * 128 + C], BF, tag="brows")
        nc.gpsimd.dma_start(brows_sb[:], brows[:])
        bcol_sb = sb_w.tile([128, 3], F32, tag="bcol")
        nc.gpsimd.dma_start(bcol_sb[:], bcol[:])
        wqkv_sb = []
        xw0 = []
        for kk in range(NK):
            t = sb_w.tile([128, 3 * 128], BF, tag=f"wqkv{kk}")
            nc.sync.dma_start(t[:], wqkv[128 * kk:128 * (kk + 1), :])
            wqkv_sb.append(t)
            tx = sb_x.tile([128, 512], BF, tag=f"xw{kk}", name=f"xw{kk}_0")
            nc.sync.dma_start(tx[:], xT[128 * kk:128 * (kk + 1), 0:512])
            xw0.append(tx)
        mask_sb = sb_w.tile([128, 4 * 512], BF, tag="mask")
        nc.sync.dma_start(mask_sb[:], masks[:])

        ones_sb = sb_w.tile([1, 512], BF, tag="ones")
        nc.vector.memset(ones_sb[:], 1.0)
        warm = sb_w.tile([1, 8], F32, tag="warm")
        nc.scalar.activation(warm[:], ones_sb[:, 0:8], ActF.Exp)

        # PE warm-up: ~3.5us of matmuls on the identity so the HAM clock
        # gate opens during the initial x-window DMA wait.
        warm_ps = ps_mm.tile([128, 512], F32, tag="mm", name="warmps")
        for i in range(28):
            nc.tensor.matmul(warm_ps[:, 0:128], ident_sb[:], ident_sb[:],
                             start=(i == 0), stop=(i == 27))

        # proj bias broadcast to all 128 rows, built once; fused into the
        # proj PSUM evacuation adds
        bias_bcast = sb_w.tile([128, C], BF, tag="bbc")
        for nn in range(C // 512):
            bps = ps_mm.tile([128, 512], F32, tag="mm", name=f"bbc{nn}")
            nc.tensor.matmul(bps[:], ones_sb[:, 0:128],
                             brows_sb[:, 384 + 512 * nn:384 + 512 * (nn + 1)],
                             start=True, stop=True)
            nc.vector.tensor_copy(bias_bcast[:, 512 * nn:512 * (nn + 1)], bps[:])

        # qT/kT/vT: [128 (=2 heads x 64 features), 4096 tokens]
        qT_sb = sb_qk.tile([128, TT], BF, tag="qT")
        kT_sb = sb_qk.tile([128, TT], BF, tag="kT")
        vT_sb = sb_qk.tile([128, TT], BF, tag="vT")
        dests = [qT_sb, kT_sb, vT_sb]
        # v natural: [128 tokens, 32 tiles x 130] = [vA(64) | 1 | vB(64) | 1]
        v_all = sb_v.tile([128, (TT // 128) * 130], BF, tag="v")
        # fill once: the per-slot ones columns survive the DMA-transposed
        # v writes, which only cover the 64-wide v blocks
        nc.vector.memset(v_all[:], 1.0)

        # ---- QKV projection, streamed over token windows ----
        def qkv_window(n):
            if n == 0:
                xw = xw0
            else:
                xw = []
                for kk in range(NK):
                    t = sb_x.tile([128, 512], BF, tag=f"xw{kk}",
                                  name=f"xw{kk}_{n}")
                    nc.sync.dma_start(t[:], xT[128 * kk:128 * (kk + 1),
                                               512 * n:512 * (n + 1)])
                    xw.append(t)
            for m in range(3):
                ps = ps_mm.tile([128, 512], F32, tag="mm", name=f"qkvps{n}_{m}")
                for kk in range(NK):
                    nc.tensor.matmul(
                        ps[:], wqkv_sb[kk][:, 128 * m:128 * (m + 1)], xw[kk][:],
                        start=(kk == 0), stop=(kk == NK - 1))
                # evacuate PSUM with the per-feature bias fused in
                nc.vector.tensor_scalar(
                    dests[m][:, 512 * n:512 * (n + 1)], ps[:],
                    bcol_sb[:, m:m + 1], None,
                    mybir.AluOpType.add)
            # transpose v window into natural layout
            for tt in range(4 * n, 4 * (n + 1)):
                tp = ps_mm.tile([128, 128], BF, tag="mm", name=f"vtp{tt}")
                nc.tensor.transpose(tp[:], vT_sb[:, 128 * tt:128 * (tt + 1)],
                                    ident_sb[:])
                nc.vector.tensor_copy(
                    v_all[:, 130 * tt:130 * (tt + 1)].rearrange(
                        "p (h c) -> p h c", c=65)[:, :, 0:64],
                    tp[:].rearrange("p (h c) -> p h c", c=64))

        # ---- attention ----
        yT_sb = sb_y.tile([128, TT], BF, tag="yT")
        # Chunked AllToAll: one collective per window-group so the exchange
        # overlaps attention compute.  Core c owns token slice
        # [64c, 64(c+1)) of EVERY 512-token window; cc buffers are
        # [8 dst cores x 128 feats, 64 tokens] per window.
        CC_GROUPS = [(0, 1), (2, 3), (4, 5), (6,), (7,)]
        cc_in = [dram.tile([NCORES * 128, 64], BF, tag=f"ccin{w}",
                           name=f"ccin{w}") for w in range(NW)]
        cc_out = [dram.tile([NCORES * 128, 64], BF, tag=f"ccout{w}",
                            name=f"ccout{w}") for w in range(NW)]
        # two attT tiles, alternated across chunks; memset once so that the
        # exp-skipped (causally invalid) columns of diagonal tiles hold
        # finite stale data for the mask-multiply to zero.
        attT_tiles = [
            sb_att.tile([128, 2 * 16 * 512], BF, tag=f"attT{i}", name=f"attT{i}")
            for i in range(2)]
        # zero the exp-skipped diagonal slots (cols 0:256 of k-tiles 4j+2,
        # 4j+3) once, so they hold finite data for the mask-multiply; all
        # later chunks either overwrite them (full exp) or skip them again.
        for p in range(2):
            for j in (p, p + 2):
                for i in (2, 3):
                    kt = 4 * j + i
                    for h in range(2):
                        nc.vector.memset(
                            attT_tiles[p][:, 8192 * h + 512 * kt:
                                          8192 * h + 512 * kt + 256], 0.0)

        def attention_chunk(b, j):
            tb = b * T
            kmax = 4 * (j + 1)
            qsl = slice(tb + 512 * j, tb + 512 * (j + 1))
            qsl_hi = slice(tb + 512 * j + 256, tb + 512 * (j + 1))
            attT = attT_tiles[(4 * b + j) % 2]
            # S^T in groups of 2 k-tiles x 2 heads -> one [128, 2048] psum
            # tile; exp per group.  The last group's two upper-diagonal
            # k-tiles only compute columns 256:512 (the rest is dead).
            # A@V matmuls for group g-1 are emitted right after group g's
            # S matmuls so the PE has ready work while exp(g) drains the
            # psum tile (ps_s is single-buffered).  Masks for the diagonal
            # tiles run right after their group's exp.
            yps = [ps_y.tile([65, 512], F32, tag="y", name=f"yps{b}_{j}_{h}")
                   for h in range(2)]

            def s_group(g):
                diag = g == kmax // 2 - 1
                sps = ps_s.tile([128, 2048], F32, tag="s", name=f"sps{b}_{j}_{g}")
                for i in range(2):
                    kt = 2 * g + i
                    ksl = slice(tb + 128 * kt, tb + 128 * (kt + 1))
                    for h in range(2):
                        hsl = slice(64 * h, 64 * (h + 1))
                        if diag:
                            nc.tensor.matmul(
                                sps[:, 1024 * h + 512 * i + 256:
                                    1024 * h + 512 * (i + 1)],
                                kT_sb[hsl, ksl], qT_sb[hsl, qsl_hi],
                                start=True, stop=True)
                        else:
                            nc.tensor.matmul(
                                sps[:, 1024 * h + 512 * i:
                                    1024 * h + 512 * (i + 1)],
                                kT_sb[hsl, ksl], qT_sb[hsl, qsl],
                                start=True, stop=True)
                if diag:
                    dst = attT[:].rearrange("p (h s q) -> p h s q",
                                            h=2, q=512)[
                        :, :, 2 * g:2 * (g + 1), 256:512]
                    srcv = sps[:].rearrange("p (h s q) -> p h s q",
                                            h=2, q=512)[:, :, :, 256:512]
                    nc.scalar.activation(dst, srcv, ActF.Exp,
                                         scale=float(1.0 / np.sqrt(D)))
                else:
                    dst = attT[:].rearrange("p (h s) -> p h s", h=2)[
                        :, :, 512 * 2 * g:512 * 2 * (g + 1)]
                    nc.scalar.activation(dst, sps[:].rearrange(
                        "p (h s) -> p h s", h=2), ActF.Exp,
                        scale=float(1.0 / np.sqrt(D)))
                # masks for this group's tiles if they are diagonal-block
                for i in range(2):
                    kt = 2 * g + i
                    di = kt - 4 * j
                    if di >= 0:
                        for h in range(2):
                            a = attT[:, 8192 * h + 512 * kt:
                                     8192 * h + 512 * (kt + 1)]
                            nc.vector.tensor_mul(
                                a, a, mask_sb[:, 512 * di:512 * (di + 1)])

            def av_group(g):
                for i in range(2):
                    kt = 2 * g + i
                    for h in range(2):
                        nc.tensor.matmul(
                            yps[h][:], v_all[:, 130 * (b * 16 + kt) + 65 * h:
                                             130 * (b * 16 + kt) + 65 * (h + 1)],
                            attT[:, 8192 * h + 512 * kt:
                                 8192 * h + 512 * (kt + 1)],
                            start=(kt == 0), stop=(kt == kmax - 1))

            s_group(0)
            for g in range(1, kmax // 2):
                s_group(g)
                av_group(g - 1)
            av_group(kmax // 2 - 1)

            for h in range(2):
                ysb = sb_tmp.tile([65, 512], F32, tag="ysb", name=f"ysb{b}{j}{h}")
                nc.vector.tensor_copy(ysb[:], yps[h][:])
                ltmp = sb_tmp.tile([1, 512], F32, tag="ltmp", name=f"lt{b}{j}{h}")
                nc.vector.tensor_copy(ltmp[:], ysb[64:65, :])
                recf = sb_tmp.tile([1, 512], F32, tag="recf", name=f"rf{b}{j}{h}")
                nc.vector.reciprocal_approx_fast(recf[:], ltmp[:])
                rec = sb_tmp.tile([1, 512], BF, tag="rec", name=f"rc{b}{j}{h}")
                nc.vector.tensor_copy(rec[:], recf[:])
                bc = ps_mm.tile([64, 512], F32, tag="mm", name=f"bc{b}{j}{h}")
                nc.tensor.matmul(bc[:], ones_sb[:, 0:64], rec[:],
                                 start=True, stop=True)
                nc.vector.tensor_mul(
                    yT_sb[64 * h:64 * (h + 1), qsl], ysb[0:64, :], bc[:])
            sh = 4 * b + j
            # one DMA: dst-core block d of cc_in[sh] gets token slice d
            nc.sync.dma_start(
                cc_in[sh][:].rearrange("(d r) c -> r d c", d=NCORES),
                yT_sb[:, 512 * sh:512 * (sh + 1)].rearrange(
                    "p (d c) -> p d c", d=NCORES))

        wproj_sb = []

        def issue_a2a(gi):
            ws = CC_GROUPS[gi]
            for w in ws:
                nc.gpsimd.collective_compute(
                    "AllToAll", mybir.AluOpType.bypass,
                    replica_groups=[list(range(NCORES))],
                    ins=[cc_in[w][:]], outs=[cc_out[w][:]])

        def proj_group(gi):
            """Project the received windows of group gi.

            Output rows (local shard order): window w contributes rows
            [64w, 64(w+1)); groups cover contiguous row blocks."""
            ws = CC_GROUPS[gi]
            m = 64 * len(ws)
            r0 = 64 * ws[0]
            y_lhs = []
            for kk in range(NK):
                t = sb_tmp.tile([128, 128], BF, tag=f"ylhs{kk}",
                                name=f"ylhs{kk}_{gi}")
                for wi, w in enumerate(ws):
                    eng = nc.sync if kk % 2 == 0 else nc.scalar
                    eng.dma_start(
                        t[:, 64 * wi:64 * (wi + 1)],
                        cc_out[w][128 * kk:128 * (kk + 1), :])
                y_lhs.append(t)
            pss = []
            for nn in range(C // 512):
                ps = ps_mm.tile([128, 512], F32, tag="mm", name=f"prj{gi}_{nn}")
                pss.append(ps)
            for kk in range(NK):
                for nn in range(C // 512):
                    nc.tensor.matmul(
                        pss[nn][0:m, :], y_lhs[kk][:, 0:m],
                        wproj_sb[kk][:, 512 * nn:512 * (nn + 1)],
                        start=(kk == 0), stop=(kk == NK - 1))
            for nn in range(C // 512):
                o = sb_out.tile([128, 512], F32, tag="o", name=f"o{gi}_{nn}")
                nc.vector.tensor_add(o[0:m, :], pss[nn][0:m, :],
                                     bias_bcast[0:m, 512 * nn:512 * (nn + 1)])
                eng = nc.sync if nn % 2 == 0 else nc.scalar
                eng.dma_start(
                    out[r0:r0 + m, 512 * nn:512 * (nn + 1)], o[0:m, :])

        # batch-0 windows, then batch-0 attention interleaved (in priority)
        # with batch-1 windows, then batch-1 attention.  A2A chunks fire as
        # their windows complete (the rendezvous with late-launched cores
        # happens on the CC stream, overlapping compute); ALL proj groups
        # are emitted last so no compute engine ever blocks mid-stream on
        # collective output.
        for n in range(4):
            qkv_window(n)
            if n == 1:
                # wproj needed once proj groups start; emitted here so its
                # DMA fills bandwidth after the first x windows.
                for kk in range(NK):
                    t = sb_w.tile([128, C], BF, tag=f"wproj{kk}")
                    nc.sync.dma_start(t[:], wproj[128 * kk:128 * (kk + 1), :])
                    wproj_sb.append(t)
        for j in range(NQ):
            attention_chunk(0, j)
            # batch-1 window emitted here so its matmuls fill the PE gaps
            # left by the S->exp PSUM ping-pong
            qkv_window(4 + j)
            if j == 1:
                issue_a2a(0)
            if j == 3:
                issue_a2a(1)
        for j in range(NQ):
            attention_chunk(1, j)
            if j == 1:
                issue_a2a(2)
            if j == 2:
                issue_a2a(3)
            if j == 3:
                # a2a chunks 0-1 completed long ago (their data was
                # exchanged during batch-1 attention): project them under
                # the final attention chunk
                proj_group(0)
                proj_group(1)
                issue_a2a(4)
        for gi in range(2, len(CC_GROUPS)):
            proj_group(gi)

    nc.compile()
    return nc


def _host_inputs(x, W_qkv, b_qkv, W_proj, b_proj):
    """Shard/layout/cast inputs for each core."""
    bf = ml_dtypes.bfloat16
    xT = np.ascontiguousarray(
        x.reshape(TT, C).T).astype(bf)                    # [C, TT]
    wproj = W_proj.astype(bf)                             # [C, C]
    kk_idx = np.arange(128)[:, None]
    qq_idx = np.arange(512)[None, :]
    masks = np.concatenate(
        [(128 * i + kk_idx <= qq_idx) for i in range(4)],
        axis=1).astype(bf)                                # [128, 2048]
    ident = np.eye(128).astype(bf)

    in_maps = []
    for c in range(NCORES):
        h0 = HP * c * D
        cols = slice(h0, h0 + HP * D)                     # 128 cols
        wq = W_qkv[:, cols]
        wk = W_qkv[:, C:][:, cols]
        wv = W_qkv[:, 2 * C:][:, cols]
        wqkv = np.concatenate([wq, wk, wv], axis=1).astype(bf)   # [C, 384]
        bq3 = np.concatenate([b_qkv[cols], b_qkv[C:][cols], b_qkv[2 * C:][cols]])
        brows = np.concatenate([bq3, b_proj])[None, :].astype(bf)  # [1, 1408]
        in_maps.append({
            "xT": xT, "wqkv": wqkv, "wproj": wproj,
            "masks": masks, "brows": brows, "bcol": np.ascontiguousarray(bq3.reshape(3, 128).T).astype(np.float32),
            "ident": ident,
        })
    return in_maps


def kernel(x, W_qkv, b_qkv, W_proj, b_proj):
    global LAST_EXEC_TIME_NS, LAST_RESULTS
    x = np.asarray(x, dtype=np.float32)
    W_qkv = np.asarray(W_qkv, dtype=np.float32)
    b_qkv = np.asarray(b_qkv, dtype=np.float32)
    W_proj = np.asarray(W_proj, dtype=np.float32)
    b_proj = np.asarray(b_proj, dtype=np.float32)

    if "nc" not in _CACHE:
        _CACHE["nc"] = build_nc()
    nc = _CACHE["nc"]

    in_maps = _host_inputs(x, W_qkv, b_qkv, W_proj, b_proj)
    trace = os.environ.get("TRN_KERNEL_TRACE", "0") == "1"
    kw = {}
    if os.environ.get("TRN_KERNEL_TRACE_ALL", "0") == "1":
        kw["trace_cores"] = list(range(NCORES))
    res = run_bass_kernel_spmd(nc, in_maps, core_ids=list(range(NCORES)),
                               trace=trace, **kw)
    LAST_EXEC_TIME_NS = res.exec_time_ns
    LAST_RESULTS = res
    # core c's out rows [64w, 64(w+1)) = global rows [512w+64c, 512w+64(c+1))
    full = np.empty((TT, C), dtype=np.float32)
    for c in range(NCORES):
        oc = res.results[c]["out"]
        for w in range(NW):
            full[512 * w + 64 * c:512 * w + 64 * (c + 1)] = \
                oc[64 * w:64 * (w + 1)]
    return full.reshape(B, T, C).astype(np.float32)

